# revision 17
# baseline (speedup 1.0000x reference)
"""RNN-T JointNetwork kernel for 8 Trainium2 NeuronCores.

Math: out[b,t,u,:] = tanh(concat(fe[b,t], gd[b,u])) @ Wj + bj
with fe = f@We+be, gd = g@Wd+bd.

Since tanh acts elementwise and the concat feeds a single GEMM, the joint
GEMM factorizes exactly:
    out[b,t,u,:] = A[b,t,:] + C[b,u,:]
    A = tanh(f@We+be) @ Wj[:Dm]          (per-(b,t) row)
    C = tanh(g@Wd+bd) @ Wj[Dm:] + bj     (per-(b,u) row)
This collapses the 137-GFLOP joint GEMM into two tiny GEMMs plus a
broadcast-add, leaving the kernel bound by the 268 MB output write.

Sharding: 8 cores, core c owns (b = c//2, t-half = c%2) -> a [128,64,V]
output chunk per core (contiguous 33.5 MB).

On-core plan (all fp32):
  - fe^T[m,t] = (We.T @ f^T) via PE (f^T from PE transpose), tanh+bias on ACT
  - gd^T[m,u] likewise
  - A[t,v] (psum = tfT.T @ Wj_top), Cp[u,v] (tgT.T @ Wj_bot + 1x bj)
  - Crep[0:128,v] = Cp stacked twice (selector matmul)
  - per 128-row output tile k (= t-pair 2k,2k+1): broadcast A rows with a
    constant 32-row selector bank (32-aligned slices of A as matmul rhs),
    add Crep on DVE (half 0) / replicate Cp on PE + copy on ACT (half 1),
    DMA 512 KB contiguous per tile.
"""

import sys

sys.path.insert(0, "/opt/trn_rl_repo")

import numpy as np

import concourse.bacc as bacc
import concourse.mybir as mybir
import concourse.tile as tile
from concourse.bass_utils import run_bass_kernel_spmd
from concourse.masks import make_identity

B, T, U = 4, 256, 64
D = 512  # DE = DD = DM
V = 1024
TC = 128  # t rows per core
NCORES = 8
FP32 = mybir.dt.float32
BF16 = mybir.dt.bfloat16
TANH = mybir.ActivationFunctionType.Tanh

_cache = {}


def _build_nc():
    nc = bacc.Bacc("TRN2", target_bir_lowering=False)

    f_d = nc.dram_tensor("f_c", [TC, D], FP32, kind="ExternalInput")
    g_d = nc.dram_tensor("g_c", [U, D], FP32, kind="ExternalInput")
    We_d = nc.dram_tensor("We", [D, D], FP32, kind="ExternalInput")
    be_d = nc.dram_tensor("be", [D], FP32, kind="ExternalInput")
    Wd_d = nc.dram_tensor("Wd", [D, D], FP32, kind="ExternalInput")
    bd_d = nc.dram_tensor("bd", [D], FP32, kind="ExternalInput")
    Wj_d = nc.dram_tensor("Wj", [2 * D, V], FP32, kind="ExternalInput")
    bj_d = nc.dram_tensor("bj", [V], FP32, kind="ExternalInput")
    out_d = nc.dram_tensor("out", [TC * U, V], FP32, kind="ExternalOutput")

    with tile.TileContext(nc) as tc:
        with (
            tc.tile_pool(name="const", bufs=1) as cp,
            tc.tile_pool(name="wts", bufs=1) as wp,
        ):
            # ---- constants ----
            ident = cp.tile([128, 128], FP32, tag="ident")
            make_identity(nc, ident[:])

            # selrep[u, j] = 1 iff j%64 == u  ([I64 | I64])
            selrep = cp.tile([64, 128], FP32, tag="selrep")
            nc.gpsimd.memset(selrep[:], 0.0)
            nc.gpsimd.affine_select(
                out=selrep[:].rearrange("p (a b) -> p a b", a=2),
                in_=selrep[:].rearrange("p (a b) -> p a b", a=2),
                compare_op=mybir.AluOpType.not_equal,
                fill=1.0,
                base=0,
                pattern=[[0, 2], [-1, 64]],
                channel_multiplier=1,
            )

            # sel32[32q + t', 128i + 64jh + jl] = 1 iff t' == 2i + jh
            # (identical pattern in each 32-partition strip q). bf16: the
            # selector is 0/1 so bf16 matmuls against bf16 hi/lo terms of A
            # select exactly, at 1 col/cycle instead of fp32's multi-pass.
            sel32 = cp.tile([128, 16 * 128], BF16, tag="sel32")
            nc.gpsimd.memset(sel32[:], 0.0)
            for q in range(4):
                sl = sel32[32 * q : 32 * q + 32, :]
                nc.gpsimd.affine_select(
                    out=sl.rearrange("p (i a b) -> p i a b", i=16, a=2),
                    in_=sl.rearrange("p (i a b) -> p i a b", i=16, a=2),
                    compare_op=mybir.AluOpType.not_equal,
                    fill=1.0,
                    base=0,
                    pattern=[[-2, 16], [-1, 2], [0, 64]],
                    channel_multiplier=1,
                )

            ones1 = cp.tile([1, 64], FP32, tag="ones1")
            nc.gpsimd.memset(ones1[:], 1.0)

            # selC[p, j] = 1 iff p%64 == j%64 (selects CHL = [C_hi; C_lo], K=128)
            selC = cp.tile([128, 128], BF16, tag="selC")
            nc.gpsimd.memset(selC[:], 0.0)
            for s in range(2):
                sl = selC[64 * s : 64 * s + 64, :]
                nc.gpsimd.affine_select(
                    out=sl.rearrange("p (a b) -> p a b", a=2),
                    in_=sl.rearrange("p (a b) -> p a b", a=2),
                    compare_op=mybir.AluOpType.not_equal,
                    fill=1.0,
                    base=0,
                    pattern=[[0, 2], [-1, 64]],
                    channel_multiplier=1,
                )

            # dup_hi/dup_lo: build AHL = [Ahi(0:32); Alo(0:32); Ahi(32:64);
            # Alo(32:64)] per 64-row half. j = 64*jh2 + 32*jm + jl.
            # dup_hi[t', j] = 1 iff jm==0 and t' == 32*jh2 + jl
            # dup_lo[t', j] = 1 iff jm==1 and t' == 32*jh2 + jl
            dup_hi = cp.tile([128, 128], BF16, tag="dup_hi")
            dup_lo = cp.tile([128, 128], BF16, tag="dup_lo")
            for tile_, base in ((dup_hi, 0), (dup_lo, 64)):
                nc.gpsimd.memset(tile_[:], 0.0)
                for s in range(2):
                    sl = tile_[64 * s : 64 * s + 64, :]
                    nc.gpsimd.affine_select(
                        out=sl.rearrange("p (a b c) -> p a b c", a=2, b=2),
                        in_=sl.rearrange("p (a b c) -> p a b c", a=2, b=2),
                        compare_op=mybir.AluOpType.not_equal,
                        fill=1.0,
                        base=base,
                        pattern=[[-32, 2], [-64 if base else 64, 2], [-1, 32]],
                        channel_multiplier=1,
                    )

            # ---- persistent operands ----
            f_sb = wp.tile([TC, D], FP32, tag="f")
            g_sb = wp.tile([U, D], FP32, tag="g")
            We_sb = [wp.tile([128, D], FP32, tag=f"We{c}", name=f"We{c}") for c in range(4)]
            Wd_sb = [wp.tile([128, D], FP32, tag=f"Wd{c}", name=f"Wd{c}") for c in range(4)]
            Wj_sb = [wp.tile([128, V], FP32, tag=f"Wj{c}", name=f"Wj{c}") for c in range(8)]
            be_sb = [wp.tile([128, 1], FP32, tag=f"be{c}", name=f"be{c}") for c in range(4)]
            bd_sb = [wp.tile([128, 1], FP32, tag=f"bd{c}", name=f"bd{c}") for c in range(4)]
            bj_sb = wp.tile([1, V], FP32, tag="bj")
            fT = [wp.tile([128, TC], FP32, tag=f"fT{c}", name=f"fT{c}") for c in range(4)]
            gT = [wp.tile([128, U], FP32, tag=f"gT{c}", name=f"gT{c}") for c in range(4)]
            tfT = [wp.tile([128, TC], FP32, tag=f"tfT{c}", name=f"tfT{c}") for c in range(4)]
            tgT = [wp.tile([128, U], FP32, tag=f"tgT{c}", name=f"tgT{c}") for c in range(4)]
            A_sb = wp.tile([TC, V], FP32, tag="A")
            A_hi = wp.tile([TC, V], BF16, tag="A_hi")
            A_lo = wp.tile([TC, V], BF16, tag="A_lo")
            A_tmp = wp.tile([TC, V], FP32, tag="A_tmp")
            AHL = [wp.tile([128, V], BF16, tag=f"AHL{h}", name=f"AHL{h}") for h in range(2)]
            Cp = wp.tile([U, V], FP32, tag="Cp")
            Crep = wp.tile([128, V], FP32, tag="Crep")
            CHL = wp.tile([128, V], BF16, tag="CHL")
            C_tmp = wp.tile([128, V], FP32, tag="C_tmp")

            nc.sync.dma_start(f_sb[:], f_d[:])
            nc.sync.dma_start(g_sb[:], g_d[:])
            for c in range(4):
                nc.sync.dma_start(We_sb[c][:], We_d[c * 128 : (c + 1) * 128, :])
                nc.sync.dma_start(Wd_sb[c][:], Wd_d[c * 128 : (c + 1) * 128, :])
                nc.sync.dma_start(
                    be_sb[c][:],
                    be_d[c * 128 : (c + 1) * 128].rearrange("(p o) -> p o", o=1),
                )
                nc.sync.dma_start(
                    bd_sb[c][:],
                    bd_d[c * 128 : (c + 1) * 128].rearrange("(p o) -> p o", o=1),
                )
            for c in range(8):
                nc.sync.dma_start(Wj_sb[c][:], Wj_d[c * 128 : (c + 1) * 128, :])
            nc.sync.dma_start(bj_sb[:], bj_d.rearrange("(p v) -> p v", p=1))

            # ---- prologue: A, Cp, Crep ----
            with tc.tile_pool(name="pp", bufs=4, space="PSUM") as pp:
                for c in range(4):
                    pt = pp.tile([128, 128], FP32, tag="pps")
                    nc.tensor.transpose(
                        pt[:], f_sb[:, c * 128 : (c + 1) * 128], ident[:]
                    )
                    nc.vector.tensor_copy(fT[c][:], pt[:])
                for c in range(4):
                    pt = pp.tile([128, U], FP32, tag="pps")
                    nc.tensor.transpose(
                        pt[:], g_sb[:, c * 128 : (c + 1) * 128], ident[0:64, 0:64]
                    )
                    nc.vector.tensor_copy(gT[c][:], pt[:])

                for mc in range(4):
                    ms = slice(mc * 128, (mc + 1) * 128)
                    ps = pp.tile([128, TC], FP32, tag="pps")
                    for dc in range(4):
                        nc.tensor.matmul(
                            ps[:],
                            We_sb[dc][:, ms],
                            fT[dc][:],
                            start=(dc == 0),
                            stop=(dc == 3),
                        )
                    nc.scalar.activation(
                        tfT[mc][:], ps[:], TANH, bias=be_sb[mc][:, 0:1]
                    )
                for mc in range(4):
                    ms = slice(mc * 128, (mc + 1) * 128)
                    ps = pp.tile([128, U], FP32, tag="pps")
                    for dc in range(4):
                        nc.tensor.matmul(
                            ps[:],
                            Wd_sb[dc][:, ms],
                            gT[dc][:],
                            start=(dc == 0),
                            stop=(dc == 3),
                        )
                    nc.scalar.activation(
                        tgT[mc][:], ps[:], TANH, bias=bd_sb[mc][:, 0:1]
                    )

                for vh in range(2):
                    vs = slice(vh * 512, (vh + 1) * 512)
                    ps = pp.tile([128, 512], FP32, tag="pps")
                    for mc in range(4):
                        nc.tensor.matmul(
                            ps[:],
                            tfT[mc][:],
                            Wj_sb[mc][:, vs],
                            start=(mc == 0),
                            stop=(mc == 3),
                        )
                    nc.vector.tensor_copy(A_sb[:, vs], ps[:])
                for vh in range(2):
                    vs = slice(vh * 512, (vh + 1) * 512)
                    ps = pp.tile([64, 512], FP32, tag="pps")
                    for mc in range(4):
                        nc.tensor.matmul(
                            ps[:],
                            tgT[mc][:],
                            Wj_sb[4 + mc][:, vs],
                            start=(mc == 0),
                            stop=False,
                        )
                    nc.tensor.matmul(
                        ps[:], ones1[:], bj_sb[:, vs], start=False, stop=True
                    )
                    nc.scalar.copy(Cp[:, vs], ps[:])
                for vh in range(2):
                    vs = slice(vh * 512, (vh + 1) * 512)
                    ps = pp.tile([128, 512], FP32, tag="pps")
                    nc.tensor.matmul(ps[:], selrep[:], Cp[:, vs], start=True, stop=True)
                    nc.vector.tensor_copy(Crep[:, vs], ps[:])

                # exact-ish two-term bf16 split: A = A_hi + A_lo + O(2^-17)
                nc.vector.tensor_copy(A_hi[:], A_sb[:])
                nc.vector.tensor_copy(A_tmp[:], A_hi[:])
                nc.vector.tensor_sub(A_tmp[:], A_sb[:], A_tmp[:])
                nc.vector.tensor_copy(A_lo[:], A_tmp[:])

                # AHL[h] = [Ahi(64h+0:32); Alo(same); Ahi(64h+32:64); Alo(same)]
                # via dup-selector matmuls (bf16 0/1 select, exact)
                for h in range(2):
                    hs = slice(64 * h, 64 * h + 64)
                    for vh in range(2):
                        vs = slice(vh * 512, (vh + 1) * 512)
                        ps = pp.tile([128, 512], FP32, tag="pps")
                        nc.tensor.matmul(
                            ps[:], dup_hi[hs, :], A_hi[hs, vs],
                            start=True, stop=False, tile_position=(64 * h, 0),
                        )
                        nc.tensor.matmul(
                            ps[:], dup_lo[hs, :], A_lo[hs, vs],
                            start=False, stop=True, tile_position=(64 * h, 0),
                        )
                        nc.vector.tensor_copy(AHL[h][:, vs], ps[:])

                # CHL = [C_hi(64); C_lo(64)] built from Crep's two copies
                # (all ops partition-aligned: DVE cannot cross partitions)
                nc.vector.tensor_copy(CHL[:], Crep[:])
                nc.vector.tensor_copy(C_tmp[64:128, :], CHL[64:128, :])
                nc.vector.tensor_sub(
                    C_tmp[64:128, :], Crep[64:128, :], C_tmp[64:128, :]
                )
                nc.vector.tensor_copy(CHL[64:128, :], C_tmp[64:128, :])

            # ---- main loop: 64 output tiles of [128, 1024] ----
            with (
                tc.tile_pool(name="po", bufs=3, space="PSUM") as po,
                tc.tile_pool(name="ob", bufs=4) as ob,
            ):
                for k in range(64):
                    q, i = k // 16, k % 16
                    h, r = q // 2, q % 2
                    rs = slice(64 * r, 64 * r + 64)
                    lhs_sel = sel32[rs, i * 128 : (i + 1) * 128]
                    psO = po.tile([128, V], FP32, tag="psO")
                    out_sb = ob.tile([128, V], FP32, tag="out")
                    # half 0: A (hi+lo packed, K=64) + C (K=128) on PE; ACT copy
                    nc.tensor.matmul(
                        psO[:, 0:512], lhs_sel, AHL[h][rs, 0:512],
                        start=True, stop=False, tile_position=(64 * r, 0),
                    )
                    nc.tensor.matmul(
                        psO[:, 0:512], selC[:], CHL[:, 0:512],
                        start=False, stop=True,
                    )
                    nc.scalar.copy(out_sb[:, 0:512], psO[:, 0:512])
                    # half 1: A on PE; DVE adds Crep on the way out of PSUM
                    nc.tensor.matmul(
                        psO[:, 512:1024], lhs_sel, AHL[h][rs, 512:1024],
                        start=True, stop=True, tile_position=(64 * r, 0),
                    )
                    nc.vector.tensor_add(
                        out_sb[:, 512:1024], psO[:, 512:1024], Crep[:, 512:1024]
                    )
                    nc.sync.dma_start(
                        out_d[k * 128 : (k + 1) * 128, :], out_sb[:]
                    )

    nc.compile()
    return nc


def kernel(f, g, We, be, Wd, bd, Wj, bj):
    if "nc" not in _cache:
        _cache["nc"] = _build_nc()
    nc = _cache["nc"]

    cast = lambda x: np.ascontiguousarray(np.asarray(x), dtype=np.float32)
    f, g = cast(f), cast(g)
    shared = {
        "We": cast(We), "be": cast(be), "Wd": cast(Wd), "bd": cast(bd),
        "Wj": cast(Wj), "bj": cast(bj),
    }
    in_maps = []
    for c in range(NCORES):
        b, th = c // 2, c % 2
        in_maps.append(
            {
                "f_c": np.ascontiguousarray(f[b, th * TC : (th + 1) * TC, :]),
                "g_c": np.ascontiguousarray(g[b]),
                **shared,
            }
        )
    res = run_bass_kernel_spmd(nc, in_maps, list(range(NCORES)))
    kernel._last_results = res

    out = np.empty((B, T, U, V), np.float32)
    for c in range(NCORES):
        b, th = c // 2, c % 2
        out[b, th * TC : (th + 1) * TC] = res.results[c]["out"].reshape(TC, U, V)
    return out


# revision 20
# speedup vs baseline: 1.4488x; 1.4488x over previous
"""RNN-T JointNetwork kernel for 8 Trainium2 NeuronCores.

Math: out[b,t,u,:] = tanh(concat(fe[b,t], gd[b,u])) @ Wj + bj
with fe = f@We+be, gd = g@Wd+bd.

Since tanh acts elementwise and the concat feeds a single GEMM, the joint
GEMM factorizes exactly:
    out[b,t,u,:] = A[b,t,:] + C[b,u,:]
    A = tanh(f@We+be) @ Wj[:Dm]          (per-(b,t) row)
    C = tanh(g@Wd+bd) @ Wj[Dm:] + bj     (per-(b,u) row)
This collapses the 137-GFLOP joint GEMM into two tiny GEMMs plus a
broadcast-add, leaving the kernel bound by the 268 MB output write.

Sharding: 8 cores, core c owns (b = c//2, t-half = c%2) -> a [128,64,V]
output chunk per core (contiguous 33.5 MB).

On-core plan (all fp32):
  - fe^T[m,t] = (We.T @ f^T) via PE (f^T from PE transpose), tanh+bias on ACT
  - gd^T[m,u] likewise
  - A[t,v] (psum = tfT.T @ Wj_top), Cp[u,v] (tgT.T @ Wj_bot + 1x bj)
  - Crep[0:128,v] = Cp stacked twice (selector matmul)
  - per 128-row output tile k (= t-pair 2k,2k+1): broadcast A rows with a
    constant 32-row selector bank (32-aligned slices of A as matmul rhs),
    add Crep on DVE (half 0) / replicate Cp on PE + copy on ACT (half 1),
    DMA 512 KB contiguous per tile.
"""

import sys

sys.path.insert(0, "/opt/trn_rl_repo")

import numpy as np

import concourse.bacc as bacc
import concourse.mybir as mybir
import concourse.tile as tile
from concourse.bass_utils import run_bass_kernel_spmd
from concourse.masks import make_identity

B, T, U = 4, 256, 64
D = 512  # DE = DD = DM
V = 1024
TC = 128  # t rows per core
NCORES = 8
FP32 = mybir.dt.float32
BF16 = mybir.dt.bfloat16
TANH = mybir.ActivationFunctionType.Tanh

_cache = {}


def _build_nc():
    nc = bacc.Bacc("TRN2", target_bir_lowering=False)

    f_d = nc.dram_tensor("f_c", [TC, D], FP32, kind="ExternalInput")
    g_d = nc.dram_tensor("g_c", [U, D], FP32, kind="ExternalInput")
    We_d = nc.dram_tensor("We", [D, D], FP32, kind="ExternalInput")
    be_d = nc.dram_tensor("be", [D], FP32, kind="ExternalInput")
    Wd_d = nc.dram_tensor("Wd", [D, D], FP32, kind="ExternalInput")
    bd_d = nc.dram_tensor("bd", [D], FP32, kind="ExternalInput")
    Wj_d = nc.dram_tensor("Wj", [2 * D, V], FP32, kind="ExternalInput")
    bj_d = nc.dram_tensor("bj", [V], FP32, kind="ExternalInput")
    out_d = nc.dram_tensor("out", [TC * U, V], FP32, kind="ExternalOutput")

    with tile.TileContext(nc) as tc:
        with (
            tc.tile_pool(name="const", bufs=1) as cp,
            tc.tile_pool(name="wts", bufs=1) as wp,
        ):
            # ---- constants ----
            ident = cp.tile([128, 128], FP32, tag="ident")
            make_identity(nc, ident[:])

            # selrep[u, j] = 1 iff j%64 == u  ([I64 | I64])
            selrep = cp.tile([64, 128], FP32, tag="selrep")
            nc.gpsimd.memset(selrep[:], 0.0)
            nc.gpsimd.affine_select(
                out=selrep[:].rearrange("p (a b) -> p a b", a=2),
                in_=selrep[:].rearrange("p (a b) -> p a b", a=2),
                compare_op=mybir.AluOpType.not_equal,
                fill=1.0,
                base=0,
                pattern=[[0, 2], [-1, 64]],
                channel_multiplier=1,
            )

            # sel32[32q + t', 128i + 64jh + jl] = 1 iff t' == 2i + jh
            # (identical pattern in each 32-partition strip q). bf16: the
            # selector is 0/1 so bf16 matmuls against bf16 hi/lo terms of A
            # select exactly, at 1 col/cycle instead of fp32's multi-pass.
            sel32 = cp.tile([128, 16 * 128], BF16, tag="sel32")
            nc.gpsimd.memset(sel32[:], 0.0)
            for q in range(4):
                sl = sel32[32 * q : 32 * q + 32, :]
                nc.gpsimd.affine_select(
                    out=sl.rearrange("p (i a b) -> p i a b", i=16, a=2),
                    in_=sl.rearrange("p (i a b) -> p i a b", i=16, a=2),
                    compare_op=mybir.AluOpType.not_equal,
                    fill=1.0,
                    base=0,
                    pattern=[[-2, 16], [-1, 2], [0, 64]],
                    channel_multiplier=1,
                )

            ones1 = cp.tile([1, 64], FP32, tag="ones1")
            nc.gpsimd.memset(ones1[:], 1.0)

            # dup_hi/dup_lo: build AHL = [Ahi(0:32); Alo(0:32); Ahi(32:64);
            # Alo(32:64)] per 64-row half. j = 64*jh2 + 32*jm + jl.
            # dup_hi[t', j] = 1 iff jm==0 and t' == 32*jh2 + jl
            # dup_lo[t', j] = 1 iff jm==1 and t' == 32*jh2 + jl
            dup_hi = cp.tile([128, 128], BF16, tag="dup_hi")
            dup_lo = cp.tile([128, 128], BF16, tag="dup_lo")
            for tile_, base in ((dup_hi, 0), (dup_lo, 64)):
                nc.gpsimd.memset(tile_[:], 0.0)
                for s in range(2):
                    sl = tile_[64 * s : 64 * s + 64, :]
                    nc.gpsimd.affine_select(
                        out=sl.rearrange("p (a b c) -> p a b c", a=2, b=2),
                        in_=sl.rearrange("p (a b c) -> p a b c", a=2, b=2),
                        compare_op=mybir.AluOpType.not_equal,
                        fill=1.0,
                        base=base,
                        pattern=[[-32, 2], [-64 if base else 64, 2], [-1, 32]],
                        channel_multiplier=1,
                    )

            # ---- persistent operands ----
            f_sb = wp.tile([TC, D], FP32, tag="f")
            g_sb = wp.tile([U, D], FP32, tag="g")
            We_sb = [wp.tile([128, D], FP32, tag=f"We{c}", name=f"We{c}") for c in range(4)]
            Wd_sb = [wp.tile([128, D], FP32, tag=f"Wd{c}", name=f"Wd{c}") for c in range(4)]
            Wj_sb = [wp.tile([128, V], FP32, tag=f"Wj{c}", name=f"Wj{c}") for c in range(8)]
            be_sb = [wp.tile([128, 1], FP32, tag=f"be{c}", name=f"be{c}") for c in range(4)]
            bd_sb = [wp.tile([128, 1], FP32, tag=f"bd{c}", name=f"bd{c}") for c in range(4)]
            bj_sb = wp.tile([1, V], FP32, tag="bj")
            fT = [wp.tile([128, TC], FP32, tag=f"fT{c}", name=f"fT{c}") for c in range(4)]
            gT = [wp.tile([128, U], FP32, tag=f"gT{c}", name=f"gT{c}") for c in range(4)]
            tfT = [wp.tile([128, TC], FP32, tag=f"tfT{c}", name=f"tfT{c}") for c in range(4)]
            tgT = [wp.tile([128, U], FP32, tag=f"tgT{c}", name=f"tgT{c}") for c in range(4)]
            A_sb = wp.tile([TC, V], FP32, tag="A")
            A_hi = wp.tile([TC, V], BF16, tag="A_hi")
            A_lo = wp.tile([TC, V], BF16, tag="A_lo")
            A_tmp = wp.tile([TC, V], FP32, tag="A_tmp")
            AHL = [wp.tile([128, V], BF16, tag=f"AHL{h}", name=f"AHL{h}") for h in range(2)]
            Cp = wp.tile([U, V], FP32, tag="Cp")
            Crep = wp.tile([128, V], FP32, tag="Crep")

            nc.sync.dma_start(f_sb[:], f_d[:])
            nc.sync.dma_start(g_sb[:], g_d[:])
            for c in range(4):
                nc.sync.dma_start(We_sb[c][:], We_d[c * 128 : (c + 1) * 128, :])
                nc.sync.dma_start(Wd_sb[c][:], Wd_d[c * 128 : (c + 1) * 128, :])
                nc.sync.dma_start(
                    be_sb[c][:],
                    be_d[c * 128 : (c + 1) * 128].rearrange("(p o) -> p o", o=1),
                )
                nc.sync.dma_start(
                    bd_sb[c][:],
                    bd_d[c * 128 : (c + 1) * 128].rearrange("(p o) -> p o", o=1),
                )
            for c in range(8):
                nc.sync.dma_start(Wj_sb[c][:], Wj_d[c * 128 : (c + 1) * 128, :])
            nc.sync.dma_start(bj_sb[:], bj_d.rearrange("(p v) -> p v", p=1))

            # ---- prologue: A, Cp, Crep ----
            with tc.tile_pool(name="pp", bufs=4, space="PSUM") as pp:
                for c in range(4):
                    pt = pp.tile([128, 128], FP32, tag="pps")
                    nc.tensor.transpose(
                        pt[:], f_sb[:, c * 128 : (c + 1) * 128], ident[:]
                    )
                    nc.vector.tensor_copy(fT[c][:], pt[:])
                for c in range(4):
                    pt = pp.tile([128, U], FP32, tag="pps")
                    nc.tensor.transpose(
                        pt[:], g_sb[:, c * 128 : (c + 1) * 128], ident[0:64, 0:64]
                    )
                    nc.vector.tensor_copy(gT[c][:], pt[:])

                for mc in range(4):
                    ms = slice(mc * 128, (mc + 1) * 128)
                    ps = pp.tile([128, TC], FP32, tag="pps")
                    for dc in range(4):
                        nc.tensor.matmul(
                            ps[:],
                            We_sb[dc][:, ms],
                            fT[dc][:],
                            start=(dc == 0),
                            stop=(dc == 3),
                        )
                    nc.scalar.activation(
                        tfT[mc][:], ps[:], TANH, bias=be_sb[mc][:, 0:1]
                    )
                for mc in range(4):
                    ms = slice(mc * 128, (mc + 1) * 128)
                    ps = pp.tile([128, U], FP32, tag="pps")
                    for dc in range(4):
                        nc.tensor.matmul(
                            ps[:],
                            Wd_sb[dc][:, ms],
                            gT[dc][:],
                            start=(dc == 0),
                            stop=(dc == 3),
                        )
                    nc.scalar.activation(
                        tgT[mc][:], ps[:], TANH, bias=bd_sb[mc][:, 0:1]
                    )

                for vh in range(2):
                    vs = slice(vh * 512, (vh + 1) * 512)
                    ps = pp.tile([128, 512], FP32, tag="pps")
                    for mc in range(4):
                        nc.tensor.matmul(
                            ps[:],
                            tfT[mc][:],
                            Wj_sb[mc][:, vs],
                            start=(mc == 0),
                            stop=(mc == 3),
                        )
                    nc.vector.tensor_copy(A_sb[:, vs], ps[:])
                for vh in range(2):
                    vs = slice(vh * 512, (vh + 1) * 512)
                    ps = pp.tile([64, 512], FP32, tag="pps")
                    for mc in range(4):
                        nc.tensor.matmul(
                            ps[:],
                            tgT[mc][:],
                            Wj_sb[4 + mc][:, vs],
                            start=(mc == 0),
                            stop=False,
                        )
                    nc.tensor.matmul(
                        ps[:], ones1[:], bj_sb[:, vs], start=False, stop=True
                    )
                    nc.scalar.copy(Cp[:, vs], ps[:])
                for vh in range(2):
                    vs = slice(vh * 512, (vh + 1) * 512)
                    ps = pp.tile([128, 512], FP32, tag="pps")
                    nc.tensor.matmul(ps[:], selrep[:], Cp[:, vs], start=True, stop=True)
                    nc.vector.tensor_copy(Crep[:, vs], ps[:])

                # exact-ish two-term bf16 split: A = A_hi + A_lo + O(2^-17)
                nc.vector.tensor_copy(A_hi[:], A_sb[:])
                nc.vector.tensor_copy(A_tmp[:], A_hi[:])
                nc.vector.tensor_sub(A_tmp[:], A_sb[:], A_tmp[:])
                nc.vector.tensor_copy(A_lo[:], A_tmp[:])

                # AHL[h] = [Ahi(64h+0:32); Alo(same); Ahi(64h+32:64); Alo(same)]
                # via dup-selector matmuls (bf16 0/1 select, exact)
                for h in range(2):
                    hs = slice(64 * h, 64 * h + 64)
                    for vh in range(2):
                        vs = slice(vh * 512, (vh + 1) * 512)
                        ps = pp.tile([128, 512], FP32, tag="pps")
                        nc.tensor.matmul(
                            ps[:], dup_hi[hs, :], A_hi[hs, vs],
                            start=True, stop=False, tile_position=(64 * h, 0),
                        )
                        nc.tensor.matmul(
                            ps[:], dup_lo[hs, :], A_lo[hs, vs],
                            start=False, stop=True, tile_position=(64 * h, 0),
                        )
                        nc.vector.tensor_copy(AHL[h][:, vs], ps[:])

            # ---- main loop: 64 output tiles of [128, 1024] ----
            with (
                tc.tile_pool(name="po", bufs=3, space="PSUM") as po,
                tc.tile_pool(name="ob", bufs=6) as ob,
            ):
                for k in range(64):
                    q, i = k // 16, k % 16
                    h, r = q // 2, q % 2
                    rs = slice(64 * r, 64 * r + 64)
                    lhs_sel = sel32[rs, i * 128 : (i + 1) * 128]
                    psO = po.tile([128, V], FP32, tag="psO")
                    out_sb = ob.tile([128, V], FP32, tag="out")
                    # A broadcast (hi+lo packed, K=64) on PE, one MM per bank
                    for vh in range(2):
                        vs = slice(vh * 512, (vh + 1) * 512)
                        nc.tensor.matmul(
                            psO[:, vs], lhs_sel, AHL[h][rs, vs],
                            start=True, stop=True, tile_position=(64 * r, 0),
                        )
                    # single full-width DVE add does C + the PSUM->SBUF move
                    nc.vector.tensor_add(out_sb[:], psO[:], Crep[:])
                    nc.sync.dma_start(
                        out_d[k * 128 : (k + 1) * 128, :], out_sb[:]
                    )

    nc.compile()
    return nc


def kernel(f, g, We, be, Wd, bd, Wj, bj):
    if "nc" not in _cache:
        _cache["nc"] = _build_nc()
    nc = _cache["nc"]

    cast = lambda x: np.ascontiguousarray(np.asarray(x), dtype=np.float32)
    f, g = cast(f), cast(g)
    shared = {
        "We": cast(We), "be": cast(be), "Wd": cast(Wd), "bd": cast(bd),
        "Wj": cast(Wj), "bj": cast(bj),
    }
    in_maps = []
    for c in range(NCORES):
        b, th = c // 2, c % 2
        in_maps.append(
            {
                "f_c": np.ascontiguousarray(f[b, th * TC : (th + 1) * TC, :]),
                "g_c": np.ascontiguousarray(g[b]),
                **shared,
            }
        )
    res = run_bass_kernel_spmd(nc, in_maps, list(range(NCORES)))
    kernel._last_results = res

    out = np.empty((B, T, U, V), np.float32)
    for c in range(NCORES):
        b, th = c // 2, c % 2
        out[b, th * TC : (th + 1) * TC] = res.results[c]["out"].reshape(TC, U, V)
    return out


# revision 21
# speedup vs baseline: 1.4858x; 1.0255x over previous
"""RNN-T JointNetwork kernel for 8 Trainium2 NeuronCores.

Math: out[b,t,u,:] = tanh(concat(fe[b,t], gd[b,u])) @ Wj + bj
with fe = f@We+be, gd = g@Wd+bd.

Since tanh acts elementwise and the concat feeds a single GEMM, the joint
GEMM factorizes exactly:
    out[b,t,u,:] = A[b,t,:] + C[b,u,:]
    A = tanh(f@We+be) @ Wj[:Dm]          (per-(b,t) row)
    C = tanh(g@Wd+bd) @ Wj[Dm:] + bj     (per-(b,u) row)
This collapses the 137-GFLOP joint GEMM into two tiny GEMMs plus a
broadcast-add, leaving the kernel bound by the 268 MB output write.

Sharding: 8 cores, core c owns (b = c//2, t-half = c%2) -> a [128,64,V]
output chunk per core (contiguous 33.5 MB).

On-core plan (all fp32):
  - fe^T[m,t] = (We.T @ f^T) via PE (f^T from PE transpose), tanh+bias on ACT
  - gd^T[m,u] likewise
  - A[t,v] (psum = tfT.T @ Wj_top), Cp[u,v] (tgT.T @ Wj_bot + 1x bj)
  - Crep[0:128,v] = Cp stacked twice (selector matmul)
  - per 128-row output tile k (= t-pair 2k,2k+1): broadcast A rows with a
    constant 32-row selector bank (32-aligned slices of A as matmul rhs),
    add Crep on DVE (half 0) / replicate Cp on PE + copy on ACT (half 1),
    DMA 512 KB contiguous per tile.
"""

import sys

sys.path.insert(0, "/opt/trn_rl_repo")

import numpy as np

import concourse.bacc as bacc
import concourse.mybir as mybir
import concourse.tile as tile
from concourse.bass_utils import run_bass_kernel_spmd
from concourse.masks import make_identity

B, T, U = 4, 256, 64
D = 512  # DE = DD = DM
V = 1024
TC = 128  # t rows per core
NCORES = 8
FP32 = mybir.dt.float32
BF16 = mybir.dt.bfloat16
TANH = mybir.ActivationFunctionType.Tanh

_cache = {}


def _build_nc():
    nc = bacc.Bacc("TRN2", target_bir_lowering=False)

    f_d = nc.dram_tensor("f_c", [TC, D], FP32, kind="ExternalInput")
    g_d = nc.dram_tensor("g_c", [U, D], FP32, kind="ExternalInput")
    We_d = nc.dram_tensor("We", [D, D], FP32, kind="ExternalInput")
    be_d = nc.dram_tensor("be", [D], FP32, kind="ExternalInput")
    Wd_d = nc.dram_tensor("Wd", [D, D], FP32, kind="ExternalInput")
    bd_d = nc.dram_tensor("bd", [D], FP32, kind="ExternalInput")
    Wj_d = nc.dram_tensor("Wj", [2 * D, V], FP32, kind="ExternalInput")
    bj_d = nc.dram_tensor("bj", [V], FP32, kind="ExternalInput")
    out_d = nc.dram_tensor("out", [TC * U, V], FP32, kind="ExternalOutput")

    with tile.TileContext(nc) as tc:
        with (
            tc.tile_pool(name="const", bufs=1) as cp,
            tc.tile_pool(name="wts", bufs=1) as wp,
        ):
            # ---- constants ----
            ident = cp.tile([128, 128], FP32, tag="ident")
            make_identity(nc, ident[:])

            # selrep[u, j] = 1 iff j%64 == u  ([I64 | I64])
            selrep = cp.tile([64, 128], FP32, tag="selrep")
            nc.gpsimd.memset(selrep[:], 0.0)
            nc.gpsimd.affine_select(
                out=selrep[:].rearrange("p (a b) -> p a b", a=2),
                in_=selrep[:].rearrange("p (a b) -> p a b", a=2),
                compare_op=mybir.AluOpType.not_equal,
                fill=1.0,
                base=0,
                pattern=[[0, 2], [-1, 64]],
                channel_multiplier=1,
            )

            # sel32[32q + t', 128i + 64jh + jl] = 1 iff t' == 2i + jh
            # (identical pattern in each 32-partition strip q). bf16: the
            # selector is 0/1 so bf16 matmuls against bf16 hi/lo terms of A
            # select exactly, at 1 col/cycle instead of fp32's multi-pass.
            sel32 = cp.tile([128, 16 * 128], BF16, tag="sel32")
            nc.gpsimd.memset(sel32[:], 0.0)
            for q in range(4):
                sl = sel32[32 * q : 32 * q + 32, :]
                nc.gpsimd.affine_select(
                    out=sl.rearrange("p (i a b) -> p i a b", i=16, a=2),
                    in_=sl.rearrange("p (i a b) -> p i a b", i=16, a=2),
                    compare_op=mybir.AluOpType.not_equal,
                    fill=1.0,
                    base=0,
                    pattern=[[-2, 16], [-1, 2], [0, 64]],
                    channel_multiplier=1,
                )

            ones1 = cp.tile([1, 64], FP32, tag="ones1")
            nc.gpsimd.memset(ones1[:], 1.0)

            # dup_hi/dup_lo: build AHL = [Ahi(0:32); Alo(0:32); Ahi(32:64);
            # Alo(32:64)] per 64-row half. j = 64*jh2 + 32*jm + jl.
            # dup_hi[t', j] = 1 iff jm==0 and t' == 32*jh2 + jl
            # dup_lo[t', j] = 1 iff jm==1 and t' == 32*jh2 + jl
            dup_hi = cp.tile([128, 128], BF16, tag="dup_hi")
            dup_lo = cp.tile([128, 128], BF16, tag="dup_lo")
            for tile_, base in ((dup_hi, 0), (dup_lo, 64)):
                nc.gpsimd.memset(tile_[:], 0.0)
                for s in range(2):
                    sl = tile_[64 * s : 64 * s + 64, :]
                    nc.gpsimd.affine_select(
                        out=sl.rearrange("p (a b c) -> p a b c", a=2, b=2),
                        in_=sl.rearrange("p (a b c) -> p a b c", a=2, b=2),
                        compare_op=mybir.AluOpType.not_equal,
                        fill=1.0,
                        base=base,
                        pattern=[[-32, 2], [-64 if base else 64, 2], [-1, 32]],
                        channel_multiplier=1,
                    )

            # ---- persistent operands ----
            f_sb = wp.tile([TC, D], FP32, tag="f")
            g_sb = wp.tile([U, D], FP32, tag="g")
            We_sb = [wp.tile([128, D], FP32, tag=f"We{c}", name=f"We{c}") for c in range(4)]
            Wd_sb = [wp.tile([128, D], FP32, tag=f"Wd{c}", name=f"Wd{c}") for c in range(4)]
            Wj_sb = [wp.tile([128, V], FP32, tag=f"Wj{c}", name=f"Wj{c}") for c in range(8)]
            be_sb = [wp.tile([128, 1], FP32, tag=f"be{c}", name=f"be{c}") for c in range(4)]
            bd_sb = [wp.tile([128, 1], FP32, tag=f"bd{c}", name=f"bd{c}") for c in range(4)]
            bj_sb = wp.tile([1, V], FP32, tag="bj")
            fT = [wp.tile([128, TC], FP32, tag=f"fT{c}", name=f"fT{c}") for c in range(4)]
            gT = [wp.tile([128, U], FP32, tag=f"gT{c}", name=f"gT{c}") for c in range(4)]
            tfT = [wp.tile([128, TC], FP32, tag=f"tfT{c}", name=f"tfT{c}") for c in range(4)]
            tgT = [wp.tile([128, U], FP32, tag=f"tgT{c}", name=f"tgT{c}") for c in range(4)]
            A_sb = wp.tile([TC, V], FP32, tag="A")
            A_hi = wp.tile([TC, V], BF16, tag="A_hi")
            A_lo = wp.tile([TC, V], BF16, tag="A_lo")
            A_tmp = wp.tile([TC, V], FP32, tag="A_tmp")
            AHL = [wp.tile([128, V], BF16, tag=f"AHL{h}", name=f"AHL{h}") for h in range(2)]
            Cp = wp.tile([U, V], FP32, tag="Cp")
            Crep = wp.tile([128, V], FP32, tag="Crep")

            nc.sync.dma_start(f_sb[:], f_d[:])
            nc.sync.dma_start(g_sb[:], g_d[:])
            for c in range(4):
                nc.sync.dma_start(We_sb[c][:], We_d[c * 128 : (c + 1) * 128, :])
                nc.sync.dma_start(Wd_sb[c][:], Wd_d[c * 128 : (c + 1) * 128, :])
                nc.sync.dma_start(
                    be_sb[c][:],
                    be_d[c * 128 : (c + 1) * 128].rearrange("(p o) -> p o", o=1),
                )
                nc.sync.dma_start(
                    bd_sb[c][:],
                    bd_d[c * 128 : (c + 1) * 128].rearrange("(p o) -> p o", o=1),
                )
            for c in range(8):
                nc.sync.dma_start(Wj_sb[c][:], Wj_d[c * 128 : (c + 1) * 128, :])
            nc.sync.dma_start(bj_sb[:], bj_d.rearrange("(p v) -> p v", p=1))

            # ---- prologue: A, Cp, Crep ----
            with tc.tile_pool(name="pp", bufs=4, space="PSUM") as pp:
                for c in range(4):
                    pt = pp.tile([128, 128], FP32, tag="pps")
                    nc.tensor.transpose(
                        pt[:], f_sb[:, c * 128 : (c + 1) * 128], ident[:]
                    )
                    nc.vector.tensor_copy(fT[c][:], pt[:])
                for c in range(4):
                    pt = pp.tile([128, U], FP32, tag="pps")
                    nc.tensor.transpose(
                        pt[:], g_sb[:, c * 128 : (c + 1) * 128], ident[0:64, 0:64]
                    )
                    nc.vector.tensor_copy(gT[c][:], pt[:])

                for mc in range(4):
                    ms = slice(mc * 128, (mc + 1) * 128)
                    ps = pp.tile([128, TC], FP32, tag="pps")
                    for dc in range(4):
                        nc.tensor.matmul(
                            ps[:],
                            We_sb[dc][:, ms],
                            fT[dc][:],
                            start=(dc == 0),
                            stop=(dc == 3),
                        )
                    nc.scalar.activation(
                        tfT[mc][:], ps[:], TANH, bias=be_sb[mc][:, 0:1]
                    )
                for mc in range(4):
                    ms = slice(mc * 128, (mc + 1) * 128)
                    ps = pp.tile([128, U], FP32, tag="pps")
                    for dc in range(4):
                        nc.tensor.matmul(
                            ps[:],
                            Wd_sb[dc][:, ms],
                            gT[dc][:],
                            start=(dc == 0),
                            stop=(dc == 3),
                        )
                    nc.scalar.activation(
                        tgT[mc][:], ps[:], TANH, bias=bd_sb[mc][:, 0:1]
                    )

                for vh in range(2):
                    vs = slice(vh * 512, (vh + 1) * 512)
                    ps = pp.tile([128, 512], FP32, tag="pps")
                    for mc in range(4):
                        nc.tensor.matmul(
                            ps[:],
                            tfT[mc][:],
                            Wj_sb[mc][:, vs],
                            start=(mc == 0),
                            stop=(mc == 3),
                        )
                    nc.vector.tensor_copy(A_sb[:, vs], ps[:])
                for vh in range(2):
                    vs = slice(vh * 512, (vh + 1) * 512)
                    ps = pp.tile([64, 512], FP32, tag="pps")
                    for mc in range(4):
                        nc.tensor.matmul(
                            ps[:],
                            tgT[mc][:],
                            Wj_sb[4 + mc][:, vs],
                            start=(mc == 0),
                            stop=False,
                        )
                    nc.tensor.matmul(
                        ps[:], ones1[:], bj_sb[:, vs], start=False, stop=True
                    )
                    nc.scalar.copy(Cp[:, vs], ps[:])
                for vh in range(2):
                    vs = slice(vh * 512, (vh + 1) * 512)
                    ps = pp.tile([128, 512], FP32, tag="pps")
                    nc.tensor.matmul(ps[:], selrep[:], Cp[:, vs], start=True, stop=True)
                    nc.vector.tensor_copy(Crep[:, vs], ps[:])

                # exact-ish two-term bf16 split A = A_hi + A_lo + O(2^-17),
                # done per 64-row half so AHL[0] (tiles 0..31) unblocks early;
                # AHL[h] = [Ahi(64h+0:32); Alo(same); Ahi(64h+32:64); Alo(same)]
                # via dup-selector matmuls (bf16 0/1 select, exact)
                for h in range(2):
                    hs = slice(64 * h, 64 * h + 64)
                    nc.vector.tensor_copy(A_hi[hs, :], A_sb[hs, :])
                    nc.vector.tensor_copy(A_tmp[hs, :], A_hi[hs, :])
                    nc.vector.tensor_sub(A_tmp[hs, :], A_sb[hs, :], A_tmp[hs, :])
                    nc.vector.tensor_copy(A_lo[hs, :], A_tmp[hs, :])
                    for vh in range(2):
                        vs = slice(vh * 512, (vh + 1) * 512)
                        ps = pp.tile([128, 512], FP32, tag="pps")
                        nc.tensor.matmul(
                            ps[:], dup_hi[hs, :], A_hi[hs, vs],
                            start=True, stop=False, tile_position=(64 * h, 0),
                        )
                        nc.tensor.matmul(
                            ps[:], dup_lo[hs, :], A_lo[hs, vs],
                            start=False, stop=True, tile_position=(64 * h, 0),
                        )
                        nc.vector.tensor_copy(AHL[h][:, vs], ps[:])

            # ---- main loop: 64 output tiles of [128, 1024] ----
            with (
                tc.tile_pool(name="po", bufs=4, space="PSUM") as po,
                tc.tile_pool(name="ob", bufs=8) as ob,
            ):
                for k in range(64):
                    q, i = k // 16, k % 16
                    h, r = q // 2, q % 2
                    rs = slice(64 * r, 64 * r + 64)
                    lhs_sel = sel32[rs, i * 128 : (i + 1) * 128]
                    psO = po.tile([128, V], FP32, tag="psO")
                    out_sb = ob.tile([128, V], FP32, tag="out")
                    # A broadcast (hi+lo packed, K=64) on PE, one MM per bank
                    for vh in range(2):
                        vs = slice(vh * 512, (vh + 1) * 512)
                        nc.tensor.matmul(
                            psO[:, vs], lhs_sel, AHL[h][rs, vs],
                            start=True, stop=True, tile_position=(64 * r, 0),
                        )
                    # single full-width DVE add does C + the PSUM->SBUF move
                    nc.vector.tensor_add(out_sb[:], psO[:], Crep[:])
                    nc.sync.dma_start(
                        out_d[k * 128 : (k + 1) * 128, :], out_sb[:]
                    )

    nc.compile()
    return nc


def kernel(f, g, We, be, Wd, bd, Wj, bj):
    if "nc" not in _cache:
        _cache["nc"] = _build_nc()
    nc = _cache["nc"]

    cast = lambda x: np.ascontiguousarray(np.asarray(x), dtype=np.float32)
    f, g = cast(f), cast(g)
    shared = {
        "We": cast(We), "be": cast(be), "Wd": cast(Wd), "bd": cast(bd),
        "Wj": cast(Wj), "bj": cast(bj),
    }
    in_maps = []
    for c in range(NCORES):
        b, th = c // 2, c % 2
        in_maps.append(
            {
                "f_c": np.ascontiguousarray(f[b, th * TC : (th + 1) * TC, :]),
                "g_c": np.ascontiguousarray(g[b]),
                **shared,
            }
        )
    res = run_bass_kernel_spmd(nc, in_maps, list(range(NCORES)))
    kernel._last_results = res

    out = np.empty((B, T, U, V), np.float32)
    for c in range(NCORES):
        b, th = c // 2, c % 2
        out[b, th * TC : (th + 1) * TC] = res.results[c]["out"].reshape(TC, U, V)
    return out
